# revision 22
# baseline (speedup 1.0000x reference)
"""Trainium2 Bass kernel for nn_Attention (B=4, N=2048, DIM=512, H=8).

Sharding: 8 cores = (batch b, seq-half s). Each core computes attention
outputs for queries [s*1024, (s+1)*1024) of batch b, all 8 heads, plus
the output projection for those rows. Outputs are disjoint -> host
gather is a pure concatenation. Keys are permuted per core (own
seq-half first) so the query chunk is always columns [0, NQ) of the
permuted x.T; attention is permutation-invariant over keys.

Performance structure (v4):
  - The PE only reaches full clock when it runs gap-free (observed:
    back-to-back matmuls hit ~215ns/512-row stream, but any
    per-iteration dependency stall drops it to ~2x slower). So the
    QKV-projection matmuls for later pairs are interleaved as *filler*
    into the attention loop, keeping the PE busy while ACT (exp) is
    the per-iteration limiter.
  - pv matmuls are software-pipelined PV_DELAY kt behind scores so
    phat is ready when the PE reaches them.
  - softmax denominators come free from a ones column appended to each
    head's V stationary (M=65): no separate sums matmuls.
  - U+sums are evacuated promptly to SBUF so the single o_ps PSUM slot
    (2 banks) recycles; PSUM = st 2x2 + o_ps 1x2 + stage 1x2 = 8 banks.
  - engines: ACT = exp + v evacuations; DVE = mask-muls, q/k/U evacs,
    reciprocal, bias; Pool = uhat norm-muls (SBUF-only) + DMA queues.
  - all inputs bf16 (halves input DMA), output bf16.

Per-core dataflow:
  q_T [512,1024]  = (SCALE*wq) @ x_chunk.T    (features x queries)
  k_T [512,2048]  = wk @ x.T                  (features x keys)
  v   [2048, 520] = x @ wv.T  (+ ones col per head, stride-65 layout)
  per head h, key-tile kt:
       scores_T[k, q] = k_h @ q_h.T          (K=64, PE quadrant po)
       p = exp(scores_T) * exp(mask).T       (mask add via exp-multiply)
       U_aug[65, q] += v_h_aug.T @ p         (PSUM accum; row 64 = sums)
  uhat_h = U * (1/sums broadcast via DMA)
  out[q,:] = Uhat.T @ proj_w.T + bias
"""
import functools
import numpy as np
import ml_dtypes
from contextlib import ExitStack

import concourse.bass as bass
import concourse.tile as tile
from concourse import bacc, mybir
from concourse.bass_utils import run_bass_kernel_spmd

F32 = mybir.dt.float32
BF16 = mybir.dt.bfloat16
AF = mybir.ActivationFunctionType

B, N, DIM, H, D = 4, 2048, 512, 8, 64
SCALE = D ** -0.5
NQ = N // 2          # queries per core
NKT = N // 128       # key tiles (16)
NCORES = 8
PV_DELAY = 4


def build(dbg=False):
    nc = bacc.Bacc("TRN2", target_bir_lowering=False, debug=False,
                   num_devices=NCORES)
    xT = nc.dram_tensor("xT", [DIM, N], BF16, kind="ExternalInput").ap()
    wqT = nc.dram_tensor("wqT", [DIM, DIM], BF16, kind="ExternalInput").ap()
    wkT = nc.dram_tensor("wkT", [DIM, DIM], BF16, kind="ExternalInput").ap()
    wvT = nc.dram_tensor("wvT", [DIM, DIM], BF16, kind="ExternalInput").ap()
    projT = nc.dram_tensor("projT", [DIM, DIM], BF16, kind="ExternalInput").ap()
    biasb = nc.dram_tensor("biasb", [128, DIM], F32, kind="ExternalInput").ap()
    expmT = nc.dram_tensor("expmT", [N, NQ], BF16, kind="ExternalInput").ap()
    out = nc.dram_tensor("out", [NQ, DIM], BF16, kind="ExternalOutput").ap()
    if dbg:
        usb00d = nc.dram_tensor("usb00d", [D + 1, NQ], F32,
                                kind="ExternalOutput").ap()
        usb01d = nc.dram_tensor("usb01d", [D + 1, NQ], F32,
                                kind="ExternalOutput").ap()
        uhat0d = nc.dram_tensor("uhat0d", [128, NQ], BF16,
                                kind="ExternalOutput").ap()
        v0d = nc.dram_tensor("v0d", [128, H * (D + 1)], BF16,
                             kind="ExternalOutput").ap()
        q0d = nc.dram_tensor("q0d", [128, NQ], BF16,
                             kind="ExternalOutput").ap()
        k0d = nc.dram_tensor("k0d", [128, N], BF16,
                             kind="ExternalOutput").ap()

    with tile.TileContext(nc) as tc, ExitStack() as ctx:
        # ---- SBUF pools ----
        wp = ctx.enter_context(tc.tile_pool(name="wp", bufs=1))
        small = ctx.enter_context(tc.tile_pool(name="small", bufs=3))
        usb_p = ctx.enter_context(tc.tile_pool(name="usb", bufs=2))
        osb = ctx.enter_context(tc.tile_pool(name="osb", bufs=2))
        praw_p = ctx.enter_context(tc.tile_pool(name="praw", bufs=3))
        phat_p = ctx.enter_context(
            tc.tile_pool(name="phat", bufs=PV_DELAY + 2))
        # ---- PSUM pools: 3x2 + 1x2 = 8 banks ----
        ps_big = ctx.enter_context(
            tc.tile_pool(name="ps_big", bufs=3, space="PSUM"))
        ps_o = ctx.enter_context(
            tc.tile_pool(name="ps_o", bufs=1, space="PSUM"))
        ps_stage = ps_big

        # ---- persistent tiles ----
        pj_sb = [wp.tile([128, DIM], BF16, name=f"pj{kc}", tag=f"pj{kc}")
                 for kc in range(4)]
        bias_sb = wp.tile([128, DIM], F32, name="bias_sb", tag="bias_sb")
        x_sb = [wp.tile([128, N], BF16, name=f"x{kc}", tag=f"x{kc}")
                for kc in range(4)]
        wq_sb = [wp.tile([128, DIM], BF16, name=f"wq{kc}", tag=f"wq{kc}")
                 for kc in range(4)]
        wk_sb = [wp.tile([128, DIM], BF16, name=f"wk{kc}", tag=f"wk{kc}")
                 for kc in range(4)]
        wv_sb = [wp.tile([128, DIM], BF16, name=f"wv{kc}", tag=f"wv{kc}")
                 for kc in range(4)]
        q_sb = [wp.tile([128, NQ], BF16, name=f"q{m}", tag=f"q{m}")
                for m in range(4)]
        k_sb = [wp.tile([128, N], BF16, name=f"k{m}", tag=f"k{m}")
                for m in range(4)]
        v_sb = [wp.tile([128, H * (D + 1)], BF16, name=f"v{kt}", tag=f"v{kt}")
                for kt in range(NKT)]
        em_sb = [wp.tile([128, NQ], BF16, name=f"em{kt}", tag=f"em{kt}")
                 for kt in range(NKT)]
        uhat = [wp.tile([128, NQ], BF16, name=f"uh{p}", tag=f"uh{p}")
                for p in range(4)]

        ones1 = wp.tile([1, 64], BF16, name="ones1", tag="ones1")
        nc.vector.memset(ones1[:], 1.0)
        # ones columns (col 64 of each head's 65-wide block)
        for kt in range(NKT):
            nc.gpsimd.memset(
                v_sb[kt].rearrange("p (h c) -> p h c", c=D + 1)[:, :, D:D + 1],
                1.0)

        # ---- input DMAs (FIFO per queue; x + weights first) ----
        for kc in range(4):
            sl = slice(kc * 128, (kc + 1) * 128)
            nc.sync.dma_start(x_sb[kc][:, 0:1024], xT[sl, 0:1024])
            nc.scalar.dma_start(wq_sb[kc][:], wqT[sl, :])
        for kc in range(4):
            sl = slice(kc * 128, (kc + 1) * 128)
            nc.sync.dma_start(x_sb[kc][:, 1024:2048], xT[sl, 1024:2048])
            nc.scalar.dma_start(wk_sb[kc][:], wkT[sl, :])
            nc.scalar.dma_start(wv_sb[kc][:], wvT[sl, :])
        for kc in range(4):
            nc.gpsimd.dma_start(pj_sb[kc][:], projT[kc * 128:(kc + 1) * 128, :])
        nc.gpsimd.dma_start(bias_sb[:], biasb[:])
        for kt in range(NKT):
            nc.scalar.dma_start(
                em_sb[kt][:], expmT[kt * 128:(kt + 1) * 128, :])

        # ---- projection-stage emitters ----
        def q_stage(m, pool):
            # two independent [128,512] column-groups, each 4mm + evac
            ms = slice(m * 128, (m + 1) * 128)
            steps = []
            for c in range(2):
                cs = slice(c * 512, (c + 1) * 512)
                cell = {}

                def mm(cell=cell, cs=cs, c=c):
                    ps = pool.tile([128, 512], F32, name=f"psq{m}_{c}",
                                   tag="big")
                    cell["ps"] = ps
                    for k2 in range(4):
                        nc.tensor.matmul(ps[:], wq_sb[k2][:, ms],
                                         x_sb[k2][:, cs],
                                         start=(k2 == 0), stop=(k2 == 3))
                steps.append(mm)

                def fin(cell=cell, cs=cs):
                    nc.vector.tensor_copy(q_sb[m][:, cs], cell["ps"][:])
                steps.append(fin)
            return steps

        def k_stage(m, half, pool):
            # two independent [128,512] column-groups, each 4mm + evac
            ms = slice(m * 128, (m + 1) * 128)
            steps = []
            for c2 in range(2):
                cs_x = slice(half * 1024 + c2 * 512,
                             half * 1024 + (c2 + 1) * 512)
                cell = {}

                def mm(cell=cell, cs_x=cs_x, c2=c2):
                    ps = pool.tile([128, 512], F32,
                                   name=f"psk{m}_{half}_{c2}", tag="big")
                    cell["ps"] = ps
                    for k2 in range(4):
                        nc.tensor.matmul(ps[:], wk_sb[k2][:, ms],
                                         x_sb[k2][:, cs_x],
                                         start=(k2 == 0), stop=(k2 == 3))
                steps.append(mm)

                def fin(cell=cell, cs_x=cs_x):
                    nc.vector.tensor_copy(k_sb[m][:, cs_x], cell["ps"][:])
                steps.append(fin)
            return steps

        def v_stage(kt):
            ks = slice(kt * 128, (kt + 1) * 128)
            ps = ps_stage.tile([128, DIM], F32, name=f"psv{kt}", tag="big")
            steps = []
            for kc in range(0, 4, 2):
                def mm(kc=kc):
                    for k2 in (kc, kc + 1):
                        nc.tensor.matmul(ps[:], x_sb[k2][:, ks], wv_sb[k2][:],
                                         start=(k2 == 0), stop=(k2 == 3))
                steps.append(mm)

            def fin():
                nc.scalar.activation(
                    v_sb[kt].rearrange(
                        "p (h c) -> p h c", c=D + 1)[:, :, 0:D],
                    ps[:].rearrange("p (h c) -> p h c", c=D), AF.Copy)
            steps.append(fin)
            return steps

        # ---- lead-in: q0, k0 (ps_big), v0..3 (ps_stage) ----
        for st_ in q_stage(0, ps_big):
            st_()
        for half in range(2):
            for st_ in k_stage(0, half, ps_big):
                st_()
        v_done = -1
        for kt in range(4):
            for st_ in v_stage(kt):
                st_()
            v_done = kt

        # ---- filler inventory ----
        # per (pair, hi): list of filler step-lists to drain during that head
        v_groups = [v_stage for _ in range(0)]  # placeholder
        filler = {}
        filler[(0, 0)] = []            # v4..15 drained JIT inside pair0.h0
        filler[(0, 1)] = ([("q", 1)] + [("k", 1, 0), ("k", 1, 1)])
        filler[(1, 0)] = [("q", 2), ("k", 2, 0)]
        filler[(1, 1)] = [("k", 2, 1), ("q", 3)]
        filler[(2, 0)] = [("k", 3, 0), ("k", 3, 1)]
        filler[(2, 1)] = []
        filler[(3, 0)] = []
        filler[(3, 1)] = []

        def make_steps(spec):
            if spec[0] == "q":
                return q_stage(spec[1], ps_stage)
            return k_stage(spec[1], spec[2], ps_stage)

        # ---- phase 2: attention with filler ----
        pending_norm = []
        o_ps_cur = {}

        def emit_pv(pending_pv):
            vs, phat, key, start, stop = pending_pv.pop(0)
            if start:
                pair, hi = key
                o_ps_cur[key] = ps_o.tile([D + 1, NQ], F32,
                                          name=f"o{pair}_{hi}", tag="o")
            o_ps = o_ps_cur[key]
            for c in range(2):
                cs = slice(c * 512, (c + 1) * 512)
                nc.tensor.matmul(o_ps[:, cs], vs, phat[:, cs],
                                 start=start, stop=stop)

        def norm_step1(key):
            # evacuate U+sums to SBUF (frees the o_ps slot), reciprocal.
            # NOTE: sums must land at partition 0 — reciprocal_approx_fast
            # reads from a partition-offset row give garbage on HW.
            pair, hi = key
            o_ps = o_ps_cur.pop(key)
            ssum = small.tile([1, NQ], F32, name=f"ss{pair}_{hi}", tag="ss")
            nc.vector.tensor_copy(ssum[:], o_ps[D:D + 1, :])
            srow = small.tile([1, NQ], F32, name=f"sr{pair}_{hi}", tag="sr")
            nc.vector.reciprocal_approx_fast(srow[:], ssum[:])
            usb = usb_p.tile([D, NQ], F32, name=f"u{pair}_{hi}",
                             tag="u")
            nc.vector.tensor_copy(usb[:], o_ps[0:D, :])
            bc = small.tile([64, NQ], F32, name=f"bc{pair}_{hi}", tag="bc")
            srcb = srow[0:1, :].rearrange(
                "p (o f) -> p o f", o=1).broadcast_to([1, 64, NQ])
            nc.sync.dma_start(bc[:], srcb)
            return (key, usb, bc)

        def norm_step2(st2):
            (pair, hi), usb, bc = st2
            nc.gpsimd.tensor_mul(
                uhat[pair][hi * 64:(hi + 1) * 64, :], usb[0:D, :], bc[:])
            if dbg and pair == 0:
                nc.sync.dma_start(
                    usb00d[0:D, :] if hi == 0 else usb01d[0:D, :], usb[:])
                if hi == 1:
                    nc.sync.dma_start(uhat0d[:], uhat[0][:])
                    nc.sync.dma_start(v0d[:], v_sb[0][:])
                    nc.sync.dma_start(q0d[:], q_sb[0][:])
                    nc.sync.dma_start(k0d[:], k_sb[0][:])

        pending_pv = []
        steps_q = []          # active filler steps for current head
        norm1 = []            # heads awaiting norm_step1
        norm2 = []            # step1 results awaiting norm_step2
        for pair in range(4):
            for hi in range(2):
                h = 2 * pair + hi
                po = hi * 64
                pos = slice(po, po + 64)
                key = (pair, hi)
                for spec in filler[key]:
                    steps_q.append(make_steps(spec))
                for kt in range(NKT):
                    # norm pipeline for the previous head
                    if kt == 4 and norm1:
                        norm2.append(norm_step1(norm1.pop(0)))
                    if kt == 8 and norm2:
                        norm_step2(norm2.pop(0))
                    # filler: JIT v stages in pair0.h0, then paced q/k
                    if key == (0, 0):
                        while v_done < min(kt + 3, NKT - 1):
                            for st_ in v_stage(v_done + 1):
                                st_()
                            v_done += 1
                    else:
                        budget = 1
                        while budget > 0 and steps_q:
                            steps_q[0].pop(0)()
                            if not steps_q[0]:
                                steps_q.pop(0)
                            budget -= 1
                    # scores
                    kts = slice(kt * 128, (kt + 1) * 128)
                    st = ps_big.tile([128, NQ], F32,
                                     name=f"st{pair}_{hi}_{kt}", tag="big")
                    for c in range(2):
                        cs = slice(c * 512, (c + 1) * 512)
                        nc.tensor.matmul(
                            st[:, cs], k_sb[pair][pos, kts],
                            q_sb[pair][pos, cs],
                            start=True, stop=True, tile_position=(po, 0))
                    praw = praw_p.tile([128, NQ], BF16,
                                       name=f"pr{pair}_{hi}_{kt}", tag="pr")
                    nc.scalar.activation(praw[:], st[:], AF.Exp)
                    phat = phat_p.tile([128, NQ], BF16,
                                       name=f"ph{pair}_{hi}_{kt}", tag="ph")
                    nc.vector.tensor_mul(phat[:], praw[:], em_sb[kt][:])
                    vs = v_sb[kt][:, h * (D + 1):(h + 1) * (D + 1)]
                    pending_pv.append((vs, phat, key, kt == 0, kt == NKT - 1))
                    if len(pending_pv) > PV_DELAY:
                        emit_pv(pending_pv)
                # drain any leftover filler before next head's scores
                # (next pair needs its q/k ready; same-pair heads share q/k)
                if hi == 1:
                    while steps_q:
                        steps_q[0].pop(0)()
                        if not steps_q[0]:
                            steps_q.pop(0)
                norm1.append(key)
        while pending_pv:
            emit_pv(pending_pv)
        while norm2:
            norm_step2(norm2.pop(0))
        # last head: bc via K=1 ones-matmul on PE (the broadcast DMA takes
        # ~11us end-to-end and would serialize the tail)
        pair, hi = norm1.pop(0)
        o_ps = o_ps_cur.pop((pair, hi))
        ssum = small.tile([1, NQ], F32, name="ss_tail", tag="ss")
        nc.vector.tensor_copy(ssum[:], o_ps[D:D + 1, :])
        srow = small.tile([1, NQ], F32, name="sr_tail", tag="sr")
        nc.vector.reciprocal_approx_fast(srow[:], ssum[:])
        srow_bf = small.tile([1, NQ], BF16, name="srbf_tail", tag="srbf")
        nc.vector.tensor_copy(srow_bf[:], srow[:])
        usb = usb_p.tile([D, NQ], F32, name="u_tail", tag="u")
        nc.vector.tensor_copy(usb[:], o_ps[0:D, :])
        bc_ps = ps_o.tile([64, NQ], F32, name="bc_ps", tag="o")
        for c in range(2):
            cs = slice(c * 512, (c + 1) * 512)
            nc.tensor.matmul(bc_ps[:, cs], ones1[:], srow_bf[0:1, cs],
                             start=True, stop=True)
        nc.vector.tensor_mul(
            uhat[pair][hi * 64:(hi + 1) * 64, :], usb[:], bc_ps[:])

        # ---- phase 3: output projection ----
        for m in range(8):
            ms = slice(m * 128, (m + 1) * 128)
            pp = ps_big.tile([128, DIM], F32, name=f"pp{m}", tag="big")
            for kc in range(4):
                nc.tensor.matmul(pp[:], uhat[kc][:, ms], pj_sb[kc][:],
                                 start=(kc == 0), stop=(kc == 3))
            ob = osb.tile([128, DIM], BF16, name=f"ob{m}", tag="ob")
            nc.vector.tensor_add(ob[:], pp[:], bias_sb[:])
            [nc.sync, nc.gpsimd][m % 2].dma_start(out[ms, :], ob[:])

    nc.compile()
    return nc


@functools.lru_cache(maxsize=1)
def _get_nc():
    return build()


def _prep_inputs(x, attn_mask, qkv_w, proj_w, proj_b):
    bf = ml_dtypes.bfloat16
    x = np.asarray(x, dtype=np.float32)
    mask = np.asarray(attn_mask, dtype=np.float32).reshape(N, N)
    qkv_w = np.asarray(qkv_w, dtype=np.float32)
    proj_w = np.asarray(proj_w, dtype=np.float32)
    proj_b = np.asarray(proj_b, dtype=np.float32)

    wqT = np.ascontiguousarray((qkv_w[0:DIM] * SCALE).T).astype(bf)
    wkT = np.ascontiguousarray(qkv_w[DIM:2 * DIM].T).astype(bf)
    wvT = np.ascontiguousarray(qkv_w[2 * DIM:3 * DIM].T).astype(bf)
    projT = np.ascontiguousarray(proj_w.T).astype(bf)
    biasb = np.tile(proj_b, (128, 1)).astype(np.float32)

    expm = np.exp(mask)
    # per-core key permutation: own seq-half first, other half second, so
    # the query chunk is always columns [0, NQ) of the permuted x.T
    xTs = {}
    emTs = {}
    for s in range(2):
        o = 1 - s
        emT = np.ascontiguousarray(expm[s * NQ:(s + 1) * NQ, :].T)  # [keys, q]
        emTs[s] = np.concatenate(
            [emT[s * NQ:(s + 1) * NQ], emT[o * NQ:(o + 1) * NQ]], axis=0
        ).astype(bf)
        for b in range(B):
            xTb = x[b].T  # [DIM, N]
            xTs[(b, s)] = np.ascontiguousarray(np.concatenate(
                [xTb[:, s * NQ:(s + 1) * NQ], xTb[:, o * NQ:(o + 1) * NQ]],
                axis=1)).astype(bf)

    in_maps = []
    for c in range(NCORES):
        b, s = c // 2, c % 2
        in_maps.append({
            "xT": xTs[(b, s)],
            "wqT": wqT, "wkT": wkT, "wvT": wvT, "projT": projT,
            "biasb": biasb, "expmT": emTs[s],
        })
    return in_maps


def run(inputs, trace=False, tmpdir=None):
    nc = _get_nc()
    in_maps = _prep_inputs(**inputs)
    res = run_bass_kernel_spmd(nc, in_maps, core_ids=list(range(NCORES)),
                               trace=trace, tmpdir=tmpdir)
    full = np.empty((B, N, DIM), dtype=np.float32)
    for c in range(NCORES):
        b, s = c // 2, c % 2
        full[b, s * NQ:(s + 1) * NQ, :] = np.asarray(
            res.results[c]["out"], dtype=np.float32)
    return full, res


def kernel(**inputs) -> np.ndarray:
    return run(inputs)[0]
